# revision 38
# baseline (speedup 1.0000x reference)
"""Trainium2 Bass kernel for the ConcreteLayer training forward pass.

Computes out = x @ softmax((weight - ln(-ln((1-tiny)*uniform + tiny))) / T, axis=1)

Strategy (8 NeuronCores, pure data-parallel, ZERO cross-core traffic):
  - Every core redundantly computes the full softmax; the batch dim is
    sharded 8 ways for the GEMM.  No collectives, no cross-core sync --
    immune to the 20-80us multi-core launch skew of this runtime.
  - T == 1 fast path (host-dispatched on the actual T value):
        e = exp(w/T) * exp(g/T),  g = -ln(-ln u')
        for T == 1:  e~ = exp(w) * (1/ln(u'))   (both factors negative
        of the true values; the signs cancel against the row-sum recip)
    which needs only TWO activation passes (Ln, Exp) instead of three;
    the reciprocal/product/row-sum run on the vector engine.
  - weight is shipped as bf16 (logit noise ~4e-3, well within the 2e-2
    budget) and x as bf16, cutting the DMA floor to ~30 MB/core.
  - Per k-tile software pipeline: DMA -> ACT(Ln,Exp) -> DVE(recip, mul,
    row-sum, normalize) -> PE (8 matmuls into PSUM), all streams overlap.
"""

import sys

import numpy as np

for _p in ("/opt/trn_rl_repo",):
    if _p not in sys.path:
        sys.path.insert(0, _p)

B, IN, OUT = 4096, 4096, 1024
NCORES = 8
BS = B // NCORES  # 512 batch rows per core
P = 128
KT = IN // P  # 32 contraction k-tiles
MBT = BS // P  # 4 output row tiles per core
NH = OUT // 512  # moving-dim chunks per matmul rhs
TINY = float(np.finfo(np.float32).tiny)

_PROGRAMS = {}
LAST_RESULT = None


def _pin_act_tables():
    """Steer the act-table-load pass to one set (has both Ln and Exp) so the
    compiler emits one ACT_TABLE_LOAD instead of reloading per tile."""
    import concourse.mybir as mybir
    from concourse import bacc, hw_specs

    orig = hw_specs.get_activation_tables.__wrapped__
    target = "natural_log_exp_and_others"
    strip = {
        mybir.ActivationFunctionType.Ln,
        mybir.ActivationFunctionType.Exp,
    }

    def pinned(arch):
        tables = orig(arch)
        if target not in tables:
            return tables
        return {
            name: (set(fns) if name == target else {f for f in fns if f not in strip})
            for name, fns in tables.items()
        }

    bacc.get_activation_tables = pinned


def _build_program(fast_t):
    import concourse.bass as bass
    import concourse.mybir as mybir
    import concourse.tile as tile
    from concourse import bacc
    from contextlib import ExitStack

    _pin_act_tables()

    f32 = mybir.dt.float32
    bf16 = mybir.dt.bfloat16
    Ln = mybir.ActivationFunctionType.Ln
    Exp = mybir.ActivationFunctionType.Exp

    nc = bacc.Bacc(
        "TRN2", target_bir_lowering=False, debug=False, num_devices=NCORES
    )

    # host-pretiled: xt_d[p, g*BS + b] = xT[g*128 + p, b]
    xt_d = nc.dram_tensor("xt", [P, KT * BS], bf16, kind="ExternalInput")
    import os as _os
    _WF32 = bool(_os.environ.get("K_W_F32"))
    wh_d = nc.dram_tensor("wh", [IN, OUT], f32 if _WF32 else bf16, kind="ExternalInput")
    uh_d = nc.dram_tensor("uh", [IN, OUT], f32, kind="ExternalInput")
    t_d = nc.dram_tensor("tt", [1], f32, kind="ExternalInput")
    out_d = nc.dram_tensor("out", [BS, OUT], f32, kind="ExternalOutput")

    with tile.TileContext(nc) as tc, ExitStack() as ctx:
        singles = ctx.enter_context(tc.tile_pool(name="singles", bufs=1))
        chunks = ctx.enter_context(tc.tile_pool(name="chunks", bufs=6))
        ep = ctx.enter_context(tc.tile_pool(name="ep", bufs=4))
        outp = ctx.enter_context(tc.tile_pool(name="outp", bufs=2))
        psum = ctx.enter_context(tc.tile_pool(name="psum", bufs=1, space="PSUM"))

        # 1/T broadcast to all partitions.
        t_sb = singles.tile([P, 1], f32)
        t_ap = t_d.ap()
        nc.sync.dma_start(
            out=t_sb, in_=bass.AP(tensor=t_ap.tensor, offset=0, ap=[[0, P], [1, 1]])
        )
        invt = singles.tile([P, 1], f32)
        nc.vector.reciprocal(invt, t_sb)
        ninvt = singles.tile([P, 1], f32)
        nc.vector.tensor_scalar_mul(ninvt, invt, -1.0)

        zero_t = singles.tile([P, 1], f32)
        nc.vector.memset(zero_t, 0.0)
        tiny_t = singles.tile([P, 1], f32)
        nc.vector.memset(tiny_t, TINY)

        # Resident xT (bf16, host-pretiled), loaded in 4 contiguous chunks
        # on the ACT HWDGE ring so the wu loads keep sync-ring priority.
        xt_all = singles.tile([P, KT, BS], bf16)

        def load_xt_quarter(qtr):
            nc.scalar.dma_start(
                out=xt_all[:, qtr * 8 : (qtr + 1) * 8, :],
                in_=xt_d[:, qtr * 8 * BS : (qtr + 1) * 8 * BS],
            )

        ps_tiles = [
            psum.tile([P, 512], f32, tag=f"ps{i}", name=f"ps{i}")
            for i in range(MBT * NH)
        ]
        # GEMM rhs must not live in a recycled pool: the PE's reads are
        # the last consumers and pool reuse races them; keep all 32
        # normalized k-tiles resident instead.
        e_all = singles.tile([P, KT, OUT], bf16)

        def ktile(g):
            u_t = chunks.tile([P, OUT], f32, tag="u", name="u_t")
            w_t = chunks.tile([P, OUT], f32 if _WF32 else bf16, tag="w", name="w_t")
            nc.sync.dma_start(out=u_t, in_=uh_d[g * P : (g + 1) * P, :])
            nc.sync.dma_start(out=w_t, in_=wh_d[g * P : (g + 1) * P, :])
            e_t = e_all[:, g, :]
            sums = chunks.tile([P, 1], f32, tag="sums", name="sums", bufs=2)
            rsum = chunks.tile([P, 1], f32, tag="rsum", name="rsum", bufs=2)
            if fast_t:
                # v = ln((1-tiny)u + tiny) (negative); negate (the fast
                # reciprocal's bit-trick seed requires positive inputs)
                nc.scalar.activation(u_t, u_t, Ln, bias=tiny_t[:], scale=1.0 - TINY)
                nc.vector.tensor_scalar_mul(u_t, u_t, -1.0)
                rv = chunks.tile([P, OUT], f32, tag="rv", name="rv", bufs=3)
                import os as _os
                if _os.environ.get("K_EXACT_RECIP"):
                    nc.vector.reciprocal(rv, u_t)
                else:
                    nc.vector.reciprocal_approx_fast(rv, u_t)
                # p = exp(w) (T==1); e~ = p * rv = exp(w)/(-ln u') on the
                # otherwise-idle gpsimd engine
                pf = chunks.tile([P, OUT], f32, tag="pf", name="pf", bufs=3)
                nc.scalar.activation(pf, w_t, Exp, bias=zero_t[:], scale=invt[:])
                et_r = chunks.tile([P, OUT], f32, tag="etr", name="et_r", bufs=3)
                nc.vector.tensor_mul(et_r, pf, rv)
                nc.vector.tensor_reduce(
                    sums, et_r, axis=mybir.AxisListType.X, op=mybir.AluOpType.add
                )
                nc.vector.reciprocal(rsum, sums)
                nc.vector.tensor_scalar_mul(e_t, et_r, rsum)
            else:
                # general T: v=ln(u'); m=ln(-v); d=w-m; e=exp(d/T) (+accum)
                nc.scalar.activation(u_t, u_t, Ln, bias=tiny_t[:], scale=1.0 - TINY)
                nc.scalar.activation(u_t, u_t, Ln, bias=zero_t[:], scale=-1.0)
                nc.vector.tensor_sub(u_t, w_t, u_t)
                nc.scalar.activation(
                    e_t, u_t, Exp, bias=zero_t[:], scale=invt[:], accum_out=sums
                )
                nc.vector.reciprocal(rsum, sums)
                nc.vector.tensor_scalar_mul(e_t, e_t, rsum)
            for mb in range(MBT):
                for h in range(NH):
                    nc.tensor.matmul(
                        ps_tiles[mb * NH + h][:],
                        lhsT=xt_all[:, g, mb * P : (mb + 1) * P],
                        rhs=e_t[:, h * 512 : (h + 1) * 512],
                        start=(g == 0),
                        stop=(g == KT - 1),
                    )

        load_xt_quarter(0)
        for g in range(KT):
            ktile(g)
            if g % 8 == 7 and g < KT - 1:
                load_xt_quarter(g // 8 + 1)

        for mb in range(MBT):
            o_t = outp.tile([P, OUT], f32, tag="o")
            for h in range(NH):
                nc.vector.tensor_copy(
                    o_t[:, h * 512 : (h + 1) * 512], ps_tiles[mb * NH + h][:]
                )
            nc.sync.dma_start(out=out_d[mb * P : (mb + 1) * P, :], in_=o_t)

    nc.compile()
    return nc


def kernel(x, weight, uniform, T):
    global LAST_RESULT
    import ml_dtypes
    from concourse.bass_utils import run_bass_kernel_spmd

    x = np.asarray(x, dtype=np.float32)
    weight = np.asarray(weight, dtype=np.float32)
    uniform = np.ascontiguousarray(np.asarray(uniform, dtype=np.float32))
    T = np.ascontiguousarray(np.asarray(T, dtype=np.float32)).reshape([1])

    import os as _os
    fast_t = bool(float(T[0]) == 1.0) and not _os.environ.get("K_FORCE_GENERAL")
    if fast_t not in _PROGRAMS:
        _PROGRAMS[fast_t] = _build_program(fast_t)
    nc = _PROGRAMS[fast_t]

    import os as _os
    if _os.environ.get("K_W_F32"):
        w16 = np.ascontiguousarray(weight)
    else:
        w16 = np.ascontiguousarray(weight.astype(ml_dtypes.bfloat16))
    xt = np.ascontiguousarray(x.T).astype(ml_dtypes.bfloat16)  # [IN, B] bf16
    in_maps = []
    for c in range(NCORES):
        xt_slice = xt[:, c * BS : (c + 1) * BS]
        xt_tiled = np.ascontiguousarray(
            xt_slice.reshape(KT, P, BS).transpose(1, 0, 2).reshape(P, KT * BS)
        )
        in_maps.append({"xt": xt_tiled, "wh": w16, "uh": uniform, "tt": T})

    res = run_bass_kernel_spmd(nc, in_maps, core_ids=list(range(NCORES)))
    LAST_RESULT = res

    out = np.empty((B, OUT), dtype=np.float32)
    for c in range(NCORES):
        out[c * BS : (c + 1) * BS, :] = res.results[c]["out"]
    return out
